# revision 31
# baseline (speedup 1.0000x reference)
"""Causal self-attention on 8 Trainium2 NeuronCores.

Sharding: batch x head-half. Core c owns batch b = c//2 and heads
8*(c%2) .. 8*(c%2)+8 (512 of the 1024 feature dims). Each core computes
QKV projections for its heads over its batch's 2048 tokens, full causal
attention for those heads, and a row-parallel partial of the output
projection. The host sums the 2 partials per batch element.

vs. the head-only sharding this reads 4x less x and writes 4x less
output per core (8.4 MB each way instead of 17/34 MB).

Layout strategy (contraction-dim-on-partitions):
  - x fed transposed: xT [C, T]; held fully resident in SBUF
  - qT, kT computed as [hd, t] ([128, hdc, t], head h = chunk h//2,
    partition half h%2)
  - v computed directly in [t, hd] layout (lhsT = x chunk) -- no PE
    transposes needed -- with a ones-column per head for softmax sums
  - ST tile = S^T = k @ q^T in [t_k, t_q] layout so softmaxed P^T is
    directly the rhs of the PV matmul; S/exp restricted to the causal
    column range [lo:512]
  - softmax denominator via the appended ones-column in the PV lhsT
  - 1/sum via reciprocal_approx_fast on the [1,512] sums row, then one
    K=2 f32r selector matmul broadcasts both heads' rows across their
    64-partition ranges
  - matmul data in fp16 (full PE rate); exp biased by -2 so fp16 never
    overflows (bias cancels exactly in softmax)
Engine split: PE matmuls only; scalar engine exp only; DVE reciprocals
+ normalize muls; Pool(gpsimd) bias-adds, causal masks, psum->sbuf
copies. S is emitted one k-chunk ahead of PV so the PE never waits on
the scalar engine's exp.
"""

import json

import numpy as np

import concourse.bass as bass
import concourse.mybir as mybir
import concourse.tile as tile
import concourse.bass2jax as bass2jax
import concourse.bass_utils as bass_utils
from concourse.bass import ts
from concourse.masks import make_upper_triangular

B, T, C, H, D = 4, 2048, 1024, 16, 64
NCORES = 8
HLOC = H // 2             # heads per core = 8
HD = HLOC * D             # local head dims = 512
NKC = C // 128            # contraction chunks for projections = 8
QB = 512                  # q block width
NQB = T // QB             # q blocks = 4
TKC = T // 128            # 128-wide k chunks = 16
NHP = HLOC // 2           # head pairs per core = 4

f32 = mybir.dt.float32
f32r = mybir.dt.float32r
f16 = mybir.dt.float16
EXP = mybir.ActivationFunctionType.Exp
EXP_BIAS = -2.0           # exp(s - 2): keeps exp outputs well inside fp16

NP16 = np.float16


# --- workaround: this walrus build accepts at most one sync wait per
# instruction; hoist surplus waits onto single-wait carriers in the BIR.
_orig_compile_bir_kernel = None

MAX_WAITS_COMPUTE = 1
MAX_WAITS_CTRL = 1


def _split_waits_in_bir(bir_json):
    d = json.loads(bir_json)
    n = 0
    for f in d.get("functions", []):
        for bb in f.get("blocks", []):
            insts = bb.get("instructions", [])
            new_insts = []
            for inst in insts:
                si = inst.get("sync_info") or {}
                waits = si.get("on_wait") or []
                limit = (
                    MAX_WAITS_CTRL
                    if inst["opcode"]
                    in ("Drain", "EventSemaphore", "NoOp", "DMACopy", "DMA")
                    else MAX_WAITS_COMPUTE
                )
                if len(waits) > limit:
                    surplus = waits[:-limit]
                    for k, w in enumerate(surplus):
                        new_insts.append({
                            "name": f"{inst['name']}_wsplit{k}",
                            "engine": inst["engine"],
                            "opcode": "EventSemaphore",
                            "ins": [],
                            "outs": [],
                            "debug": inst.get("debug", 0),
                            "sync_info": {"on_update": [], "on_wait": [w]},
                        })
                        n += 1
                    si["on_wait"] = waits[-limit:]
                    inst["sync_info"] = si
                new_insts.append(inst)
            bb["instructions"] = new_insts
    return json.dumps(d).encode()


def _install_wait_split():
    global _orig_compile_bir_kernel
    if _orig_compile_bir_kernel is not None:
        return
    _orig_compile_bir_kernel = bass2jax.compile_bir_kernel

    def _patched(bir_json, tmpdir, neff_name="file.neff"):
        return _orig_compile_bir_kernel(
            _split_waits_in_bir(bir_json), tmpdir, neff_name
        )

    bass2jax.compile_bir_kernel = _patched


def build_program():
    nc = bass.Bass()
    xT = nc.declare_dram_parameter("xT", [C, T], f16, isOutput=False)
    wqkvT = nc.declare_dram_parameter("wqkvT", [C, 3 * HD], f16, isOutput=False)
    wpT = nc.declare_dram_parameter("wpT", [HD, C], f16, isOutput=False)
    bqk = nc.declare_dram_parameter("bqk", [HD, 2], f32, isOutput=False)
    bvr = nc.declare_dram_parameter("bvr", [1, HD], f32, isOutput=False)
    outT = nc.declare_dram_parameter("outT", [C, T], f16, isOutput=True)

    with tile.TileContext(nc) as tc:
        with (
            tc.tile_pool(name="consts", bufs=1) as consts,
            tc.tile_pool(name="persist", bufs=1) as persist,
        ):
            tri = consts.tile([128, 128], f16)
            make_upper_triangular(nc, tri, val=1.0, diag=True)
            expbias = consts.tile([128, 1], f32)
            nc.vector.memset(expbias, EXP_BIAS)
            ones64 = consts.tile([1, 64], f16)
            nc.vector.memset(ones64, 1.0)
            ones1 = consts.tile([1, 128], f32)
            nc.vector.memset(ones1, 1.0)

            wq_sb = consts.tile([128, NKC, 3 * HD], f16)
            wp_sb = consts.tile([128, 4, C], f16)
            bqk_sb = consts.tile([128, 4, 2], f32)
            bv_sb = consts.tile([1, HD], f32)
            vbias = consts.tile([128, HD], f32)

            x_sb = persist.tile([128, NKC, T], f16)
            qT = persist.tile([128, 4, T], f16)
            kT = persist.tile([128, 4, T], f16)
            yT = persist.tile([128, 4, T], f16)
            # v in [t, hd] layout + a ones column per head for softmax sums
            v_sb = persist.tile([128, TKC, HLOC, 66], f16)
            nc.gpsimd.memset(v_sb[:, :, :, 64], 1.0)

            xTr = xT.rearrange("(kc p) t -> p kc t", p=128)
            wqr = wqkvT.rearrange("(kc p) n -> p kc n", p=128)
            wpr = wpT.rearrange("(hc p) c -> p hc c", p=128)

            # ---- input DMAs: tiny bias tensors first (the bias-adds gate
            # the PSUM drain), then weight/x pieces in first-use order ----
            nc.sync.dma_start(bqk_sb, bqk.rearrange("(hc p) n -> p hc n", p=128))
            nc.sync.dma_start(bv_sb, bvr[:, :])
            for kc in range(NKC):
                nc.sync.dma_start(wq_sb[:, kc, :], wqr[:, kc, :])
                nc.sync.dma_start(x_sb[:, kc, 0:QB], xTr[:, kc, 0:QB])
            for tb in range(1, 4):
                nc.sync.dma_start(x_sb[:, :, ts(tb, QB)], xTr[:, :, ts(tb, QB)])
            nc.sync.dma_start(wp_sb, wpr)

            # ---- shared pools: phase-1 chains, ypq and pp share the
            # 4-buffer "big" PSUM tag; st keeps 4 banks ----
            with (
                tc.tile_pool(name="p2", bufs=1) as p2,
                tc.tile_pool(name="ps2", bufs=1, space="PSUM") as ps2,
            ):
                def proj_mm(pp, hdc, oc, qsl):
                    nc.tensor.matmul(
                        pp,
                        lhsT=wp_sb[:, hdc, ts(oc, 128)],
                        rhs=yT[:, hdc, qsl],
                        start=(hdc == 0),
                        stop=(hdc == 3),
                    )

                def proj_out(pp, oc, qsl):
                    ob = p2.tile([128, QB], f16, tag="ob", bufs=4)
                    nc.vector.tensor_copy(ob, pp)
                    nc.sync.dma_start(outT[ts(oc, 128), qsl], ob)

                class Block:
                    """One (q-block, head-pair) attention block.

                    s/e are memoized so the scheduler can pre-emit them as
                    boundary preludes (PE/scalar filler) without
                    double-emission."""

                    def __init__(self, j, hp):
                        self.j = j
                        self.hp = hp
                        self.q_off = j * QB
                        self.qsl = slice(self.q_off, self.q_off + QB)
                        self.nkc = 4 * (j + 1)
                        self.sts = {}
                        self.exs = {}
                        self.ypq = None
                        self.sdone = set()
                        self.edone = set()

                    def s(self, kc):
                        if kc in self.sdone or kc >= self.nkc:
                            return
                        self.sdone.add(kc)
                        r = kc * 128 - self.q_off
                        lo = max(r, 0)
                        st = ps2.tile([128, 2, QB], f32, tag="st", bufs=2,
                                      name="st")
                        for s in range(2):
                            nc.tensor.matmul(
                                st[:, s, lo:QB],
                                lhsT=kT[ts(s, 64), self.hp, ts(kc, 128)],
                                rhs=qT[ts(s, 64), self.hp,
                                       self.q_off + lo:self.q_off + QB],
                                start=True,
                                stop=True,
                            )
                        self.sts[kc] = (st, r, lo)

                    def e(self, kc):
                        if kc in self.edone or kc >= self.nkc:
                            return
                        self.edone.add(kc)
                        st, r, lo = self.sts.pop(kc)
                        ex = p2.tile([128, 2, QB], f16, tag="ex", bufs=18)
                        nc.scalar.activation(
                            ex[:, :, lo:QB], st[:, :, lo:QB], EXP,
                            scale=0.125, bias=expbias,
                        )
                        if r >= 0:
                            for s in range(2):
                                nc.gpsimd.tensor_mul(
                                    ex[:, s, r:r + 128], ex[:, s, r:r + 128],
                                    tri,
                                )
                        self.exs[kc] = (ex, lo)

                    def pv(self, kc):
                        if self.ypq is None:
                            self.ypq = [
                                ps2.tile([128, QB], f32, tag="big", bufs=4,
                                         name=f"ypq{s}")
                                for s in range(2)
                            ]
                        ex, lo = self.exs.pop(kc)
                        for s in range(2):
                            nc.tensor.matmul(
                                self.ypq[s][0:65, lo:QB],
                                lhsT=v_sb[:, kc, 2 * self.hp + s, 0:65],
                                rhs=ex[:, s, lo:QB],
                                start=(kc == 0),
                                stop=(kc == self.nkc - 1),
                            )

                    def sums(self):
                        # pull the sums rows out eagerly (f16 cast) so the
                        # deferred broadcast matmul has them ready
                        self.srows = []
                        for s in range(2):
                            srow = p2.tile([1, QB], f16, tag="srow", bufs=6)
                            nc.vector.tensor_copy(srow, self.ypq[s][64:65, :])
                            self.srows.append(srow)

                    def fin(self):
                        # normalize: yT = ypq[0:64] * (1/sums). Each head's
                        # sums row is broadcast across its 64-partition range
                        # by a K=1 f16 matmul into an st-rotation bank; a
                        # fast copy frees that bank immediately and the slow
                        # reciprocal runs SBUF->SBUF off the critical path.
                        stb = ps2.tile([128, 2, QB], f32, tag="st", bufs=2,
                                       name="st")
                        bcp = stb[:, 0, :]
                        for s in range(2):
                            nc.tensor.matmul(
                                bcp[ts(s, 64), :],
                                lhsT=ones64,
                                rhs=self.srows[s],
                                start=True,
                                stop=True,
                            )
                        bcs = p2.tile([128, QB], f32, tag="bcs", bufs=3)
                        nc.vector.tensor_copy(bcs, bcp)
                        rec = p2.tile([128, QB], f32, tag="rec", bufs=3)
                        nc.vector.reciprocal(rec, bcs)
                        for s in range(2):
                            nc.vector.tensor_mul(
                                yT[ts(s, 64), self.hp, self.qsl],
                                self.ypq[s][0:64, :],
                                rec[ts(s, 64), :],
                            )

                def prelude(b):
                    b.s(0)
                    b.s(1)
                    b.e(0)
                    b.s(2)
                    b.s(3)
                    b.e(1)

                def proj(qsl):
                    pps = {}
                    for oc in range(2):
                        pps[oc] = ps2.tile([128, QB], f32, tag="big",
                                           bufs=4, name=f"pp{oc % 2}")
                        for hdc in range(3):
                            proj_mm(pps[oc], hdc, oc, qsl)
                    for oc in range(2):
                        proj_mm(pps[oc], 3, oc, qsl)
                        proj_out(pps[oc], oc, qsl)
                    for oc in range(2, 8):
                        pp = ps2.tile([128, QB], f32, tag="big",
                                      bufs=4, name=f"pp{oc % 2}")
                        for hdc in range(4):
                            proj_mm(pp, hdc, oc, qsl)
                        proj_out(pp, oc, qsl)

                blks = [Block(j, hp) for j in range(NQB) for hp in range(NHP)]

                # ---- phase 1: QKV projections ----
                # broadcast the v bias row across 128 partitions (one-time)
                psb = ps2.tile([128, HD], f32, tag="big", bufs=4)
                nc.tensor.matmul(psb, lhsT=ones1, rhs=bv_sb, start=True,
                                 stop=True)
                nc.vector.tensor_copy(vbias, psb)

                def emit_tb(tb):
                    tsl = ts(tb, QB)
                    # q and k: [hd_chunk, t] layout; emit each head pair's
                    # q chunk right before its k chunk so attention on this
                    # token block can start as early as possible
                    for pc in (0, 4, 1, 5, 2, 6, 3, 7):
                        dst, hdc = (qT, pc) if pc < 4 else (kT, pc - 4)
                        ps = ps2.tile([128, QB], f32, tag="big", bufs=4)
                        for kc in range(NKC):
                            nc.tensor.matmul(
                                ps,
                                lhsT=wq_sb[:, kc, ts(pc, 128)],
                                rhs=x_sb[:, kc, tsl],
                                start=(kc == 0),
                                stop=(kc == NKC - 1),
                            )
                        nc.vector.tensor_scalar_add(
                            dst[:, hdc, tsl], ps,
                            bqk_sb[:, hdc, (pc // 4):(pc // 4) + 1],
                        )
                    # v: [t, hd] layout directly (no transposes)
                    for i in range(4):
                        tkc = tb * 4 + i
                        psv = ps2.tile([128, HD], f32, tag="big", bufs=4)
                        for kc in range(NKC):
                            nc.tensor.matmul(
                                psv,
                                lhsT=x_sb[:, kc, ts(tkc, 128)],
                                rhs=wq_sb[:, kc, 2 * HD:3 * HD],
                                start=(kc == 0),
                                stop=(kc == NKC - 1),
                            )
                        nc.vector.tensor_add(
                            v_sb[:, tkc, :, 0:64],
                            psv.rearrange("p (h d) -> p h d", h=HLOC),
                            vbias.rearrange("p (h d) -> p h d", h=HLOC),
                        )

                state = {"fin": None}

                def pre_se(b, ne=None):
                    # emit all of b's S matmuls and exps (no PVs)
                    ne = b.nkc if ne is None else ne
                    b.s(0)
                    b.s(1)
                    for kc in range(ne):
                        b.e(kc)
                        b.s(kc + 2)

                def pop_fin():
                    if state["fin"] is not None:
                        state["fin"]()
                        state["fin"] = None

                def run_block(blk, nxt=None, do_proj=False):
                    for kc in range(blk.nkc):
                        blk.s(kc + 2)
                        blk.e(kc + 1)
                        if kc == 1:
                            pop_fin()
                        blk.pv(kc)
                    blk.sums()
                    if do_proj:
                        if nxt is not None:
                            prelude(nxt)
                        pop_fin()
                        blk.fin()
                        proj(blk.qsl)
                    else:
                        pop_fin()
                        state["fin"] = blk.fin
                        if nxt is not None:
                            prelude(nxt)

                # interleaved schedule: j0/j1 attention woven between the
                # remaining QKV token blocks so the scalar engine's exp
                # stream starts ~60us before the projections finish
                emit_tb(0)
                for hp in range(NHP):
                    pre_se(blks[hp])
                emit_tb(1)
                run_block(blks[0])
                run_block(blks[1])
                run_block(blks[2])
                run_block(blks[3])
                pre_se(blks[4])
                pre_se(blks[5])
                pop_fin()
                emit_tb(2)
                proj(blks[3].qsl)
                run_block(blks[4])
                run_block(blks[5])
                pre_se(blks[6])
                pre_se(blks[7])
                emit_tb(3)
                run_block(blks[6])
                run_block(blks[7], nxt=blks[8], do_proj=True)

                for i in range(8, len(blks)):
                    blk = blks[i]
                    nxt = blks[i + 1] if i + 1 < len(blks) else None
                    run_block(blk, nxt=nxt, do_proj=(blk.hp == NHP - 1))
    return nc


_program = None


def _get_program():
    global _program
    if _program is None:
        _install_wait_split()
        _program = build_program()
    return _program


def kernel(x, Wq, bq, Wk, bk, Wv, bv, Wp, bp):
    nc = _get_program()

    x = np.asarray(x, dtype=np.float32)
    Wqf = np.asarray(Wq, np.float32)
    Wkf = np.asarray(Wk, np.float32)
    Wvf = np.asarray(Wv, np.float32)
    Wpf = np.asarray(Wp, np.float32)

    in_maps = []
    for core in range(NCORES):
        b, half = divmod(core, 2)
        rows = slice(half * HD, (half + 1) * HD)
        xTb = np.ascontiguousarray(x[b].T.astype(NP16))
        wqkvT = np.ascontiguousarray(
            np.concatenate(
                [W[rows].T for W in (Wqf, Wkf, Wvf)], axis=1
            ).astype(NP16)
        )
        wpT = np.ascontiguousarray(Wpf[:, rows].T.astype(NP16))
        bqk_l = np.stack(
            [np.asarray(v, np.float32)[rows] for v in (bq, bk)], axis=1
        )
        bvr = np.asarray(bv, np.float32)[rows].reshape(1, HD)
        in_maps.append(
            {
                "xT": xTb,
                "wqkvT": wqkvT,
                "wpT": wpT,
                "bqk": np.ascontiguousarray(bqk_l),
                "bvr": np.ascontiguousarray(bvr),
            }
        )

    r = bass_utils.run_bass_kernel_spmd(nc, in_maps, list(range(NCORES)))
    out = np.empty((B, T, C), np.float32)
    for b in range(B):
        acc = (r.results[2 * b]["outT"].astype(np.float32)
               + r.results[2 * b + 1]["outT"].astype(np.float32))
        out[b] = acc.T
    out += np.asarray(bp, np.float32)[None, None, :]
    return out.astype(np.float32)
